# revision 2
# baseline (speedup 1.0000x reference)
"""Trainium2 Bass kernel for nn_DTMTIR_79559974191225.

Computation (reference):
    mu, ls: [K=64, T=100, V=10000] -> time-major [T, K, V]
    betas = mu + exp(0.5*ls) * eps                  (eps: [T, K, V])
    out1  = softmax(betas, axis=-1)                 [T, K, V]
    out2  = kl scalar:
        0.5 * sum[(exp(ls)+diff^2)*a_t - 1*... - ls + c_t]  with
        diff[t] = mu[t] - betas[t-1] (betas[-1] := 0),
        a_t = 1/(1+eps_div) if t==0 else 1/(delta+eps_div),
        c_t = -1 if t==0 else -1+log(delta)

Sharding: flatten rows r = t*64 + k (6400 rows). Each of 8 cores owns 800
consecutive rows; softmax over V is fully core-local (V not sharded).
The t-1 coupling (diff) needs beta of rows r-64: each core recomputes a
64-row halo locally (beta is a pure elementwise function of the inputs),
so there are no collectives at all. The scalar KL is assembled on the
host from per-(tile,partition,chunk) partial sums returned by each core.

Per-core layout: shard rows s = r - (800c - 64), s in [0, 896).
B-tiles i=0..6 process rows [128i, 128i+128). beta is stored in
"stiles": stile_m holds beta rows [128m-64, 128m+64) so that the diff
op (mu[row] - beta[row-64]) is partition-aligned; the beta compute
writes each half of a B-tile's rows into two neighboring stiles
(output base partition may differ from input base; verified on HW).
"""

import numpy as np

try:
    import concourse.bacc as bacc  # noqa: F401
except Exception:  # pragma: no cover - path fallback
    import sys

    sys.path.insert(0, "/opt/trn_rl_repo")

K = 64
T = 100
V = 10000
NCORES = 8
ROWS = T * K          # 6400
OWN = ROWS // NCORES  # 800 rows per core
SROWS = 896           # shard rows: 64 halo + 800 own + 32 overread
NB = 7                # B-tiles of 128 rows
VC = 1250             # streamed chunk width
NJ = V // VC          # 8
VC2 = 2500            # stile-op chunk width
NJ2 = V // VC2        # 4
NCOLS = NB * NJ       # 56 accumulator columns per stat
DELTA = 0.005
EPS_DIV = 1e-6

_CACHE = {}


def _build_module():
    import concourse.bacc as bacc
    import concourse.tile as tile
    import concourse.mybir as mybir

    f32 = mybir.dt.float32
    AF = mybir.ActivationFunctionType
    ALU = mybir.AluOpType
    AX = mybir.AxisListType

    nc = bacc.Bacc("TRN2", target_bir_lowering=False, debug=False)
    mu_d = nc.dram_tensor("mu", [SROWS, V], f32, kind="ExternalInput")
    ls_d = nc.dram_tensor("ls", [SROWS, V], f32, kind="ExternalInput")
    ep_d = nc.dram_tensor("ep", [SROWS, V], f32, kind="ExternalInput")
    out_d = nc.dram_tensor("probs", [OWN, V], f32, kind="ExternalOutput")
    st_d = nc.dram_tensor("stats", [128, 3 * NCOLS], f32, kind="ExternalOutput")

    with tile.TileContext(nc) as tc:
        with (
            tc.tile_pool(name="stiles", bufs=2) as stile_pool,
            tc.tile_pool(name="mu", bufs=3) as mu_pool,
            tc.tile_pool(name="ls", bufs=2) as ls_pool,
            tc.tile_pool(name="ep", bufs=3) as ep_pool,
            tc.tile_pool(name="sig", bufs=2) as sig_pool,
            tc.tile_pool(name="prod", bufs=2) as prod_pool,
            tc.tile_pool(name="diff", bufs=2) as diff_pool,
            tc.tile_pool(name="big", bufs=2) as big_pool,
            tc.tile_pool(name="accs", bufs=1) as acc_pool,
            tc.tile_pool(name="small", bufs=4) as small_pool,
            tc.tile_pool(name="psq", bufs=2, space="PSUM") as psum_pool,
        ):
            acc_s2 = acc_pool.tile([128, NCOLS], f32, tag="acc_s2")
            acc_d2 = acc_pool.tile([128, NCOLS], f32, tag="acc_d2")
            acc_ls = acc_pool.tile([128, NCOLS], f32, tag="acc_ls")
            acc_S = acc_pool.tile([128, 8 * NJ2], f32, tag="acc_S")
            nc.vector.memset(acc_s2, 0.0)
            nc.vector.memset(acc_d2, 0.0)
            nc.vector.memset(acc_ls, 0.0)
            nc.vector.memset(acc_S, 0.0)

            stiles = {}

            def get_stile(m):
                if m not in stiles:
                    stiles[m] = stile_pool.tile(
                        [128, V], f32, tag="stile", name=f"stile{m}"
                    )
                return stiles[m]

            def softmax_emit(m):
                """Softmax pass over stile m (beta rows [128m-64, 128m+64))."""
                st = stiles[m]
                p1 = 64 if m == 7 else 128  # stile 7 upper half never written
                for j2 in range(NJ2):
                    vs = slice(j2 * VC2, (j2 + 1) * VC2)
                    e_t = big_pool.tile([128, VC2], f32, tag="e")
                    nc.scalar.activation(
                        out=e_t[0:p1, :], in_=st[0:p1, vs], func=AF.Exp,
                        accum_out=acc_S[0:p1, m * NJ2 + j2 : m * NJ2 + j2 + 1],
                    )
                ssum = small_pool.tile([128, 1], f32, tag="ssum")
                nc.vector.tensor_reduce(
                    out=ssum[0:p1], in_=acc_S[0:p1, m * NJ2 : (m + 1) * NJ2],
                    axis=AX.X, op=ALU.add,
                )
                lns = small_pool.tile([128, 1], f32, tag="lns")
                nc.scalar.activation(out=lns[0:p1], in_=ssum[0:p1], func=AF.Ln)
                nneg = small_pool.tile([128, 1], f32, tag="nneg")
                nc.vector.tensor_scalar_mul(nneg[0:p1], lns[0:p1], -1.0)
                orow = 128 * (m - 1)
                pw = 32 if m == 7 else 128  # valid output rows in this stile
                for j2 in range(NJ2):
                    vs = slice(j2 * VC2, (j2 + 1) * VC2)
                    o_t = big_pool.tile([128, VC2], f32, tag="oc")
                    nc.scalar.activation(
                        out=o_t[0:p1, :], in_=st[0:p1, vs], func=AF.Exp,
                        bias=nneg[0:p1],
                    )
                    nc.sync.dma_start(
                        out=out_d[orow : orow + pw, vs], in_=o_t[0:pw, :]
                    )

            for i in range(NB):
                st_i = get_stile(i)
                st_n = get_stile(i + 1)
                r0 = 128 * i
                for j in range(NJ):
                    vs = slice(j * VC, (j + 1) * VC)
                    col = i * NJ + j
                    mu_t = mu_pool.tile([128, VC], f32, tag="mu")
                    ls_t = ls_pool.tile([128, VC], f32, tag="ls")
                    ep_t = ep_pool.tile([128, VC], f32, tag="ep")
                    nc.sync.dma_start(out=mu_t, in_=mu_d[r0 : r0 + 128, vs])
                    nc.sync.dma_start(out=ls_t, in_=ls_d[r0 : r0 + 128, vs])
                    nc.sync.dma_start(out=ep_t, in_=ep_d[r0 : r0 + 128, vs])

                    # sig = exp(0.5*ls)                          [ACT]
                    sig_t = sig_pool.tile([128, VC], f32, tag="sig")
                    nc.scalar.activation(
                        out=sig_t, in_=ls_t, func=AF.Exp, scale=0.5
                    )
                    # sum(ls) over chunk                          [DVE]
                    nc.vector.tensor_reduce(
                        out=acc_ls[:, col : col + 1], in_=ls_t, axis=AX.X,
                        op=ALU.add,
                    )
                    # sig^2 elementwise + accum sum(exp(ls))      [DVE]
                    s2s = diff_pool.tile([128, VC], f32, tag="s2s")
                    nc.vector.scalar_tensor_tensor(
                        out=s2s, in0=sig_t, scalar=1.0, in1=sig_t,
                        op0=ALU.mult, op1=ALU.mult,
                        accum_out=acc_s2[:, col : col + 1],
                    )
                    # prod = sig * eps                            [GPSIMD]
                    prod_t = prod_pool.tile([128, VC], f32, tag="prod")
                    nc.gpsimd.tensor_tensor(
                        out=prod_t, in0=sig_t, in1=ep_t, op=ALU.mult
                    )
                    # beta halves into neighboring stiles         [DVE]
                    nc.vector.tensor_tensor(
                        out=st_i[64:128, vs], in0=prod_t[0:64, :],
                        in1=mu_t[0:64, :], op=ALU.add,
                    )
                    nc.vector.tensor_tensor(
                        out=st_n[0:64, vs], in0=prod_t[64:128, :],
                        in1=mu_t[64:128, :], op=ALU.add,
                    )
                    # diff = mu - beta_prev                       [GPSIMD]
                    # sqd = diff^2 (psum scrap) + accum           [ACT]
                    diff_t = diff_pool.tile([128, VC], f32, tag="diff")
                    sq_t = psum_pool.tile([128, VC], f32, tag="sq")
                    if i == 0:
                        nc.gpsimd.tensor_tensor(
                            out=diff_t[64:128, :], in0=mu_t[64:128, :],
                            in1=st_i[64:128, vs], op=ALU.subtract,
                        )
                        nc.scalar.activation(
                            out=sq_t[64:128, :], in_=diff_t[64:128, :],
                            func=AF.Square,
                            accum_out=acc_d2[64:128, col : col + 1],
                        )
                    else:
                        nc.gpsimd.tensor_tensor(
                            out=diff_t, in0=mu_t, in1=st_i[:, vs],
                            op=ALU.subtract,
                        )
                        nc.scalar.activation(
                            out=sq_t, in_=diff_t, func=AF.Square,
                            accum_out=acc_d2[:, col : col + 1],
                        )
                if i >= 1:
                    softmax_emit(i)
            softmax_emit(7)

            nc.sync.dma_start(out=st_d[:, 0:NCOLS], in_=acc_s2)
            nc.sync.dma_start(out=st_d[:, NCOLS : 2 * NCOLS], in_=acc_d2)
            nc.sync.dma_start(out=st_d[:, 2 * NCOLS : 3 * NCOLS], in_=acc_ls)

    nc.finalize()
    return nc


def _get_module():
    if "nc" not in _CACHE:
        _CACHE["nc"] = _build_module()
    return _CACHE["nc"]


def _shard(x_rows, c):
    """Rows [800c-64, 800c+832) of a [6400, V] array, zero-padded OOB."""
    lo = OWN * c - 64
    hi = lo + SROWS
    if lo >= 0 and hi <= ROWS:
        return np.ascontiguousarray(x_rows[lo:hi])
    out = np.zeros((SROWS, V), np.float32)
    a, b = max(lo, 0), min(hi, ROWS)
    out[a - lo : b - lo] = x_rows[a:b]
    return out


def _assemble_kl(stats_list):
    a0 = 1.0 / (1.0 + EPS_DIV)
    a1 = 1.0 / (DELTA + EPS_DIV)
    c0 = -1.0
    c1 = -1.0 + float(np.log(DELTA))
    kl = 0.0
    p = np.arange(128)
    for c, st in enumerate(stats_list):
        st = st.astype(np.float64)
        s2 = st[:, 0:NCOLS]
        d2 = st[:, NCOLS : 2 * NCOLS]
        lsum = st[:, 2 * NCOLS : 3 * NCOLS]
        for i in range(NB):
            S2 = s2[:, i * NJ : (i + 1) * NJ].sum(axis=1)
            D2 = d2[:, i * NJ : (i + 1) * NJ].sum(axis=1)
            LS = lsum[:, i * NJ : (i + 1) * NJ].sum(axis=1)
            srow = 128 * i + p
            owned = (srow >= 64) & (srow < 64 + OWN)
            r = OWN * c - 64 + srow
            t = r // 64
            a = np.where(t == 0, a0, a1)
            cc = np.where(t == 0, c0, c1)
            kl += np.sum(np.where(owned, a * (S2 + D2) - LS + cc * V, 0.0))
    return 0.5 * kl


def kernel_with_results(mu_q_beta, logsig_q_beta, eps, trace=False, **run_kwargs):
    from concourse.bass_utils import run_bass_kernel_spmd

    nc = _get_module()
    mu_rows = np.transpose(np.asarray(mu_q_beta), (1, 0, 2)).reshape(ROWS, V)
    ls_rows = np.transpose(np.asarray(logsig_q_beta), (1, 0, 2)).reshape(ROWS, V)
    ep_rows = np.asarray(eps).reshape(ROWS, V)
    in_maps = [
        {"mu": _shard(mu_rows, c), "ls": _shard(ls_rows, c), "ep": _shard(ep_rows, c)}
        for c in range(NCORES)
    ]
    res = run_bass_kernel_spmd(
        nc, in_maps, core_ids=list(range(NCORES)), trace=trace, **run_kwargs
    )
    probs = np.concatenate(
        [res.results[c]["probs"] for c in range(NCORES)], axis=0
    ).reshape(T, K, V)
    kl = _assemble_kl([res.results[c]["stats"] for c in range(NCORES)])
    return (probs, np.float32(kl)), res


def kernel(mu_q_beta, logsig_q_beta, eps):
    out, _ = kernel_with_results(mu_q_beta, logsig_q_beta, eps)
    return out
